# revision 19
# baseline (speedup 1.0000x reference)
"""MoE ConditionalFeedForward (SwiGLU experts, top-k routing) on 8 TRN2 cores.

Strategy: expert parallelism with load-balanced pairing. Each of the 8 cores
owns 2 experts, chosen by sorting experts by routed-token count and pairing
k-th largest with k-th smallest, so per-core work is near the mean. Slot 0
(the heavier expert) is padded to C1 = global max count; slot 1 to
C2 = 9th-largest count. The host gathers each expert's tokens (bf16,
transposed so channels sit on the partition axis), the device computes

    y_e = (silu(x_e @ w1_e.T) * (x_e @ w3_e.T)) @ w2_e.T

in bf16 tiles (GEMM1 accumulates over DIM in PSUM, SwiGLU fused on the
ACT Silu LUT + one DVE mul, GEMM2 accumulates over INTER), and the host
applies
router combine weights and scatter-adds per-expert outputs (fp32 combine,
bf16 device output).
"""

import numpy as np
import ml_dtypes

import concourse.bacc as bacc
import concourse.mybir as mybir
import concourse.tile as tile
from concourse.bass_utils import run_bass_kernel_spmd

P = 128
T = 8192
DIM = 2560
INTER = 1664
E = 16
TOPK = 6
NCORES = 8
EPC = E // NCORES  # experts per core

BF16 = mybir.dt.bfloat16
F32 = mybir.dt.float32

NS_CHUNK = 512  # moving-dim chunk per matmul (PSUM bank = 512 fp32)
FIRST_BLOCK = 512  # first block per rep; small enough to restart fast after
                   # the loop barrier, big enough not to underrun the weight
                   # stream (a block always pulls 25.6MB of weights, so token
                   # count < ~500 demands > ~160GB/s and stalls the PE)


def even_split(C, cap):
    """Split C into ceil(C/cap) near-equal 16-multiples."""
    import math
    n = max(1, math.ceil(C / cap))
    base = (C // n) // 16 * 16
    sizes = [base] * n
    extra = C - base * n
    i = 0
    while extra > 0:
        add = min(16, extra)
        sizes[i % n] += add
        extra -= add
        i += 1
    return sizes


def make_blocks(C, max_block=1104, first_block=0):
    """Decompose capacity C into token blocks (offset, size).

    With first_block > 0 the first block is fixed-size: its x/weights sit
    in dedicated preloaded buffers so the PE restarts right after the
    per-iteration loop barrier. The rest is split evenly so no block
    drops below ~512 tokens (weight-stream underrun)."""
    blocks = []
    t0 = 0
    if first_block and C > first_block * 2:
        blocks.append((0, first_block))
        t0 = first_block
    for B in even_split(C - t0, max_block):
        blocks.append((t0, B))
        t0 += B
    return blocks


def ns_chunks(B, ns_chunk=NS_CHUNK):
    """Even moving-dim chunking: [(offset, width)] with width <= 512."""
    out = []
    off = 0
    for w in even_split(B, ns_chunk):
        out.append((off, w))
        off += w
    return out


def build_nc(caps, dim=DIM, inter=INTER, reps=1, max_block=1104,
             ns_chunk=None, timing_mode=False, loop_reps=0):
    """caps: per-expert-slot token capacities, len EPC."""
    if isinstance(caps, int):
        caps = [caps] * EPC
    if ns_chunk is None:
        ns_chunk = NS_CHUNK
    KO = dim // P     # k-chunks over model dim
    MO = inter // P   # m-chunks over intermediate dim

    nc = bacc.Bacc(None, target_bir_lowering=False)
    with tile.TileContext(nc) as tc:
        with tc.tile_pool(name="dram", bufs=1, space="DRAM") as dram:
            xts, yts = [], []
            for e in range(EPC):
                xts.append(dram.tile([P, KO, caps[e]], BF16,
                                     kind="ExternalInput", name=f"xt{e}",
                                     uniquify=False))
                yts.append(dram.tile(
                    [P, KO, caps[e]], BF16,
                    kind="Internal" if timing_mode else "ExternalOutput",
                    name=f"yt{e}", uniquify=False))
            w1t = dram.tile([EPC, MO, P, KO, P], BF16, kind="ExternalInput",
                            name="w1t", uniquify=False)
            w3t = dram.tile([EPC, MO, P, KO, P], BF16, kind="ExternalInput",
                            name="w3t", uniquify=False)
            w2t = dram.tile([EPC, KO, P, MO, P], BF16, kind="ExternalInput",
                            name="w2t", uniquify=False)
            dum = None
            if timing_mode:
                dum = dram.tile([P, P], F32, kind="ExternalOutput",
                                name="dum", uniquify=False)

            use_pre = caps[0] > FIRST_BLOCK * 2  # dedicated first-block bufs
            with (
                tc.tile_pool(name="xp", bufs=1) as xp,
                tc.tile_pool(name="xp0", bufs=1) as xp0,
                tc.tile_pool(name="wf", bufs=1) as wf,
                tc.tile_pool(name="wp", bufs=6) as wp,
                tc.tile_pool(name="w2p", bufs=4) as w2p,
                tc.tile_pool(name="gp", bufs=1) as gp,
                tc.tile_pool(name="sp", bufs=4) as sp,
                tc.tile_pool(name="op", bufs=4) as op,
                tc.tile_pool(name="ps", bufs=8, space="PSUM") as ps,
            ):
                import contextlib

                def load_first(xs0, w1f, w3f):
                    """Fill the dedicated first-block x + mo0 weight tiles."""
                    for k0 in range(0, KO, 5):
                        nc.gpsimd.dma_start(
                            xs0[:, k0:k0 + 5, :],
                            xts[0][:, k0:k0 + 5, 0:FIRST_BLOCK])
                    nc.sync.dma_start(w1f[:], w1t[0, 0])
                    nc.sync.dma_start(w3f[:], w3t[0, 0])

                def do_block(e, t0, B, xs, w13_first=None):
                    gt = gp.tile([P, MO, max_block], BF16, tag="gt")
                    # GEMM1 + SwiGLU: g.T = silu(w1.x.T) * (w3.x.T)
                    for mo in range(MO):
                        if mo == 0 and w13_first is not None:
                            w1s, w3s = w13_first
                        else:
                            w1s = wp.tile([P, KO, P], BF16, tag="w13")
                            nc.sync.dma_start(w1s[:], w1t[e, mo])
                            w3s = wp.tile([P, KO, P], BF16, tag="w13")
                            nc.sync.dma_start(w3s[:], w3t[e, mo])
                        for ns, NB in ns_chunks(B, ns_chunk):
                            p1 = ps.tile([P, ns_chunk], F32, tag="ps")
                            p3 = ps.tile([P, ns_chunk], F32, tag="ps")
                            for ko in range(KO):
                                nc.tensor.matmul(
                                    p1[:, :NB], lhsT=w1s[:, ko, :],
                                    rhs=xs[:, ko, ns:ns + NB],
                                    start=(ko == 0), stop=(ko == KO - 1))
                            for ko in range(KO):
                                nc.tensor.matmul(
                                    p3[:, :NB], lhsT=w3s[:, ko, :],
                                    rhs=xs[:, ko, ns:ns + NB],
                                    start=(ko == 0), stop=(ko == KO - 1))
                            sl = sp.tile([P, ns_chunk], F32, tag="sl")
                            nc.scalar.activation(
                                sl[:, :NB], p1[:, :NB],
                                mybir.ActivationFunctionType.Silu)
                            nc.vector.tensor_tensor(
                                gt[:, mo, ns:ns + NB], sl[:, :NB],
                                p3[:, :NB], mybir.AluOpType.mult)
                    # GEMM2: y.T = w2 . g.T  (accumulate over INTER)
                    for dd in range(KO):
                        w2s = w2p.tile([P, MO, P], BF16, tag="w2")
                        nc.sync.dma_start(w2s[:], w2t[e, dd])
                        for ns, NB in ns_chunks(B, ns_chunk):
                            po = ps.tile([P, ns_chunk], F32, tag="ps")
                            for mo in range(MO):
                                nc.tensor.matmul(
                                    po[:, :NB], lhsT=w2s[:, mo, :],
                                    rhs=gt[:, mo, ns:ns + NB],
                                    start=(mo == 0), stop=(mo == MO - 1))
                            ob = op.tile([P, ns_chunk], BF16, tag="ob")
                            nc.vector.tensor_copy(ob[:, :NB], po[:, :NB])
                            # outputs on the ACT queue so they never
                            # delay the next block's weight loads
                            nc.scalar.dma_start(
                                yts[e][:, dd, t0 + ns:t0 + ns + NB],
                                ob[:, :NB])

                xs0 = w1f = w3f = None
                if use_pre:
                    # preamble: first block's x + mo0 weights live in
                    # dedicated buffers, loaded before the timing loop and
                    # refilled mid-body, so after the For_i barrier the PE
                    # restarts with zero DMA wait.
                    xs0 = xp0.tile([P, KO, FIRST_BLOCK], BF16, tag="xs0")
                    w1f = wf.tile([P, KO, P], BF16, tag="w1f")
                    w3f = wf.tile([P, KO, P], BF16, tag="w3f")
                    load_first(xs0, w1f, w3f)

                loop_cm = (tc.For_i(0, loop_reps, 1) if loop_reps
                           else contextlib.nullcontext())
                with loop_cm:
                  for _ in range(reps):
                    for e in range(EPC):
                        first = FIRST_BLOCK if (e == 0 and use_pre) else 0
                        for (t0, B) in make_blocks(caps[e], max_block, first):
                            if e == 0 and t0 == 0 and use_pre:
                                do_block(e, t0, B, xs0, (w1f, w3f))
                                continue
                            xs = xp.tile([P, KO, max_block], BF16, tag="xs")
                            # x on its own DMA queue (gpsimd), chunked so
                            # the first chain only waits on the first chunk
                            for k0 in range(0, KO, 5):
                                nc.gpsimd.dma_start(
                                    xs[:, k0:k0 + 5, :B],
                                    xts[e][:, k0:k0 + 5, t0:t0 + B])
                            do_block(e, t0, B, xs)
                    if loop_reps and use_pre:
                        # refill the dedicated first-block buffers for the
                        # next iteration; lands well before the barrier.
                        load_first(xs0, w1f, w3f)
                if timing_mode:
                    db = op.tile([P, P], F32, tag="dumb")
                    nc.any.memset(db[:], 0.0)
                    nc.sync.dma_start(dum[:], db[:])
    nc.compile()
    return nc


def route(expert_indices, expert_weights, n_experts):
    """Per-expert token ids and merged combine weights (duplicates summed)."""
    idx = np.asarray(expert_indices)
    ew = np.asarray(expert_weights, dtype=np.float32)
    ids, wts = [], []
    for e in range(n_experts):
        m = idx == e                       # [T, K]
        rows = np.nonzero(m.any(axis=1))[0]
        ids.append(rows)
        wts.append((ew * m).sum(axis=1)[rows])
    return ids, wts


def balance(ids):
    """Pair k-th largest with k-th smallest expert by count.

    Returns (assign, caps): assign[core] = (slot0 expert, slot1 expert),
    caps = [C1, C2] padded slot capacities shared by all cores."""
    counts = np.array([len(i) for i in ids])
    order = np.argsort(-counts, kind="stable")
    assign = [(int(order[c]), int(order[E - 1 - c])) for c in range(NCORES)]
    pad = lambda n: max(((n + 15) // 16) * 16, 256)
    caps = [pad(int(counts[order[0]])), pad(int(counts[order[NCORES]]))]
    return assign, caps


def pack_weights(w1, w2, w3, dim=DIM, inter=INTER):
    """Pre-transpose weights to the device layout, cast bf16."""
    KO, MO = dim // P, inter // P
    w1b = np.asarray(w1).astype(ml_dtypes.bfloat16)
    w3b = np.asarray(w3).astype(ml_dtypes.bfloat16)
    w2b = np.asarray(w2).astype(ml_dtypes.bfloat16)
    ne = w1b.shape[0]
    # w1/w3: [e, INTER, DIM] -> [e, MO, P(k), KO, P(m)]
    w1p = w1b.reshape(ne, MO, P, KO, P).transpose(0, 1, 4, 3, 2).copy()
    w3p = w3b.reshape(ne, MO, P, KO, P).transpose(0, 1, 4, 3, 2).copy()
    # w2: [e, DIM, INTER] -> [e, KO, P(k over inter), MO, P(m over dim)]
    w2p = w2b.reshape(ne, KO, P, MO, P).transpose(0, 1, 4, 3, 2).copy()
    return w1p, w3p, w2p


def pack_tokens(x, ids, C, dim=DIM):
    """Gather routed tokens, pad to C, transpose to [P, KO, C] bf16."""
    KO = dim // P
    cnt = len(ids)
    xg = np.zeros((C, dim), dtype=np.float32)
    xg[:cnt] = np.asarray(x)[ids]
    xgb = xg.astype(ml_dtypes.bfloat16)
    return xgb.reshape(C, KO, P).transpose(2, 1, 0).copy()


def unpack_out(yt_e, C, dim=DIM):
    """[P, KO, C] -> [C, DIM] f32."""
    return yt_e.transpose(2, 1, 0).reshape(C, dim).astype(np.float32)


def kernel(x, expert_indices, expert_weights, w1, w2, w3):
    x = np.asarray(x, dtype=np.float32)
    w1 = np.asarray(w1, dtype=np.float32)
    w2 = np.asarray(w2, dtype=np.float32)
    w3 = np.asarray(w3, dtype=np.float32)

    ids, wts = route(expert_indices, expert_weights, E)
    assign, caps = balance(ids)

    nc = build_nc(caps)

    w1p, w3p, w2p = pack_weights(w1, w2, w3)
    in_maps = []
    for core in range(NCORES):
        exps = list(assign[core])
        im = {
            "w1t": w1p[exps].copy(),
            "w3t": w3p[exps].copy(),
            "w2t": w2p[exps].copy(),
        }
        for j, e in enumerate(exps):
            im[f"xt{j}"] = pack_tokens(x, ids[e], caps[j])
        in_maps.append(im)

    res = run_bass_kernel_spmd(nc, in_maps, core_ids=list(range(NCORES)))

    out = np.zeros((T, DIM), dtype=np.float32)
    for core in range(NCORES):
        for j, e in enumerate(assign[core]):
            cnt = len(ids[e])
            if cnt == 0:
                continue
            y = unpack_out(res.results[core][f"yt{j}"], caps[j])
            out[ids[e]] += wts[e][:, None] * y[:cnt]
    return out



# revision 26
# speedup vs baseline: 1.0624x; 1.0624x over previous
"""MoE ConditionalFeedForward (SwiGLU experts, top-k routing) on 8 TRN2 cores.

Strategy: expert parallelism with load-balanced pairing. Each of the 8 cores
owns 2 experts, chosen by sorting experts by routed-token count and pairing
k-th largest with k-th smallest, so per-core work is near the mean. Slot 0
(the heavier expert) is padded to C1 = global max count; slot 1 to
C2 = 9th-largest count. The host gathers each expert's tokens (bf16,
transposed so channels sit on the partition axis), the device computes

    y_e = (silu(x_e @ w1_e.T) * (x_e @ w3_e.T)) @ w2_e.T

in bf16 tiles (GEMM1 accumulates over DIM in PSUM, SwiGLU fused on the
ACT Silu LUT + one DVE mul, GEMM2 accumulates over INTER), and the host
applies
router combine weights and scatter-adds per-expert outputs (fp32 combine,
bf16 device output).
"""

import numpy as np
import ml_dtypes

import concourse.bacc as bacc
import concourse.mybir as mybir
import concourse.tile as tile
from concourse.bass_utils import run_bass_kernel_spmd

P = 128
T = 8192
DIM = 2560
INTER = 1664
E = 16
TOPK = 6
NCORES = 8
EPC = E // NCORES  # experts per core

BF16 = mybir.dt.bfloat16
F32 = mybir.dt.float32

NS_CHUNK = 512  # moving-dim chunk per matmul (PSUM bank = 512 fp32)
FIRST_BLOCK = 512  # first block per rep; small enough to restart fast after
                   # the loop barrier, big enough not to underrun the weight
                   # stream (a block always pulls 25.6MB of weights, so token
                   # count < ~500 demands > ~160GB/s and stalls the PE)


def even_split(C, cap):
    """Split C into ceil(C/cap) near-equal 16-multiples."""
    import math
    n = max(1, math.ceil(C / cap))
    base = (C // n) // 16 * 16
    sizes = [base] * n
    extra = C - base * n
    i = 0
    while extra > 0:
        add = min(16, extra)
        sizes[i % n] += add
        extra -= add
        i += 1
    return sizes


def make_blocks(C, max_block=1104, first_block=0):
    """Decompose capacity C into token blocks (offset, size).

    With first_block > 0 the first block is fixed-size: its x/weights sit
    in dedicated preloaded buffers so the PE restarts right after the
    per-iteration loop barrier. The rest is split evenly so no block
    drops below ~512 tokens (weight-stream underrun)."""
    blocks = []
    t0 = 0
    if first_block and C > first_block * 2:
        blocks.append((0, first_block))
        t0 = first_block
    for B in even_split(C - t0, max_block):
        blocks.append((t0, B))
        t0 += B
    return blocks


def ns_chunks(B, ns_chunk=NS_CHUNK):
    """Even moving-dim chunking: [(offset, width)] with width <= 512."""
    out = []
    off = 0
    for w in even_split(B, ns_chunk):
        out.append((off, w))
        off += w
    return out


def build_nc(caps, dim=DIM, inter=INTER, reps=1, max_block=1104,
             ns_chunk=None, timing_mode=False, loop_reps=0):
    """caps: per-expert-slot token capacities, len EPC."""
    if isinstance(caps, int):
        caps = [caps] * EPC
    if ns_chunk is None:
        ns_chunk = NS_CHUNK
    KO = dim // P     # k-chunks over model dim
    MO = inter // P   # m-chunks over intermediate dim

    nc = bacc.Bacc(None, target_bir_lowering=False)
    with tile.TileContext(nc) as tc:
        with tc.tile_pool(name="dram", bufs=1, space="DRAM") as dram:
            xts, yts = [], []
            for e in range(EPC):
                xts.append(dram.tile([P, KO, caps[e]], BF16,
                                     kind="ExternalInput", name=f"xt{e}",
                                     uniquify=False))
                yts.append(dram.tile(
                    [P, KO, caps[e]], BF16,
                    kind="Internal" if timing_mode else "ExternalOutput",
                    name=f"yt{e}", uniquify=False))
            w1t = dram.tile([EPC, MO, P, KO, P], BF16, kind="ExternalInput",
                            name="w1t", uniquify=False)
            w3t = dram.tile([EPC, MO, P, KO, P], BF16, kind="ExternalInput",
                            name="w3t", uniquify=False)
            w2t = dram.tile([EPC, KO, P, MO, P], BF16, kind="ExternalInput",
                            name="w2t", uniquify=False)
            dum = None
            if timing_mode:
                dum = dram.tile([P, P], F32, kind="ExternalOutput",
                                name="dum", uniquify=False)

            use_pre = caps[0] > FIRST_BLOCK * 2  # dedicated first-block bufs
            with (
                tc.tile_pool(name="xp", bufs=1) as xp,
                tc.tile_pool(name="xp0", bufs=1) as xp0,
                tc.tile_pool(name="wf", bufs=1) as wf,
                tc.tile_pool(name="wp", bufs=6) as wp,
                tc.tile_pool(name="w2p", bufs=4) as w2p,
                tc.tile_pool(name="gp", bufs=1) as gp,
                tc.tile_pool(name="sp", bufs=4) as sp,
                tc.tile_pool(name="op", bufs=4) as op,
                tc.tile_pool(name="ps", bufs=8, space="PSUM") as ps,
            ):
                import contextlib

                NPRE = 2  # prefetched leading mo's of w1/w3 and dd's of w2

                def load_first(xs0, w13f, w2f):
                    """Fill the dedicated first-block x + leading weight
                    tiles (mo 0..NPRE-1 of w1/w3, dd 0..NPRE-1 of w2), so
                    the post-barrier weight queue starts NPRE*3 DMAs
                    ahead of the PE's demand."""
                    for k0 in range(0, KO, 5):
                        nc.gpsimd.dma_start(
                            xs0[:, k0:k0 + 5, :],
                            xts[0][:, k0:k0 + 5, 0:FIRST_BLOCK])
                    for mo in range(NPRE):
                        nc.sync.dma_start(w13f[mo][0][:], w1t[0, mo])
                        nc.sync.dma_start(w13f[mo][1][:], w3t[0, mo])
                    for dd in range(NPRE):
                        nc.sync.dma_start(w2f[dd][:], w2t[0, dd])

                def do_block(e, t0, B, xs, w13_first=None, w2_first=None):
                    gt = gp.tile([P, MO, max_block], BF16, tag="gt")
                    # GEMM1 + SwiGLU: g.T = silu(w1.x.T) * (w3.x.T)
                    for mo in range(MO):
                        if w13_first is not None and mo < len(w13_first):
                            w1s, w3s = w13_first[mo]
                        else:
                            w1s = wp.tile([P, KO, P], BF16, tag="w13")
                            nc.sync.dma_start(w1s[:], w1t[e, mo])
                            w3s = wp.tile([P, KO, P], BF16, tag="w13")
                            nc.sync.dma_start(w3s[:], w3t[e, mo])
                        for ns, NB in ns_chunks(B, ns_chunk):
                            p1 = ps.tile([P, ns_chunk], F32, tag="ps")
                            p3 = ps.tile([P, ns_chunk], F32, tag="ps")
                            for ko in range(KO):
                                nc.tensor.matmul(
                                    p1[:, :NB], lhsT=w1s[:, ko, :],
                                    rhs=xs[:, ko, ns:ns + NB],
                                    start=(ko == 0), stop=(ko == KO - 1))
                            for ko in range(KO):
                                nc.tensor.matmul(
                                    p3[:, :NB], lhsT=w3s[:, ko, :],
                                    rhs=xs[:, ko, ns:ns + NB],
                                    start=(ko == 0), stop=(ko == KO - 1))
                            sl = sp.tile([P, ns_chunk], F32, tag="sl")
                            nc.scalar.activation(
                                sl[:, :NB], p1[:, :NB],
                                mybir.ActivationFunctionType.Silu)
                            nc.vector.tensor_tensor(
                                gt[:, mo, ns:ns + NB], sl[:, :NB],
                                p3[:, :NB], mybir.AluOpType.mult)
                    # GEMM2: y.T = w2 . g.T  (accumulate over INTER)
                    for dd in range(KO):
                        if w2_first is not None and dd < len(w2_first):
                            w2s = w2_first[dd]
                        else:
                            w2s = w2p.tile([P, MO, P], BF16, tag="w2")
                            nc.sync.dma_start(w2s[:], w2t[e, dd])
                        for ns, NB in ns_chunks(B, ns_chunk):
                            po = ps.tile([P, ns_chunk], F32, tag="ps")
                            for mo in range(MO):
                                nc.tensor.matmul(
                                    po[:, :NB], lhsT=w2s[:, mo, :],
                                    rhs=gt[:, mo, ns:ns + NB],
                                    start=(mo == 0), stop=(mo == MO - 1))
                            ob = op.tile([P, ns_chunk], BF16, tag="ob")
                            nc.vector.tensor_copy(ob[:, :NB], po[:, :NB])
                            # outputs on the ACT queue so they never
                            # delay the next block's weight loads
                            nc.scalar.dma_start(
                                yts[e][:, dd, t0 + ns:t0 + ns + NB],
                                ob[:, :NB])

                xs0 = w13f = w2f = None
                if use_pre:
                    # preamble: first block's x + leading weights live in
                    # dedicated buffers, loaded before the timing loop and
                    # refilled mid-body, so after the For_i barrier the PE
                    # restarts with zero DMA wait and the weight queue has
                    # a head start.
                    xs0 = xp0.tile([P, KO, FIRST_BLOCK], BF16, tag="xs0")
                    w13f, w2f = [], []
                    for m in range(NPRE):
                        w1fm = wf.tile([P, KO, P], BF16, tag=f"w1f{m}")
                        w3fm = wf.tile([P, KO, P], BF16, tag=f"w3f{m}")
                        w13f.append((w1fm, w3fm))
                    for dd in range(NPRE):
                        w2fd = wf.tile([P, MO, P], BF16, tag=f"w2f{dd}")
                        w2f.append(w2fd)
                    load_first(xs0, w13f, w2f)

                loop_cm = (tc.For_i(0, loop_reps, 1) if loop_reps
                           else contextlib.nullcontext())
                with loop_cm:
                  for _ in range(reps):
                    for e in range(EPC):
                        first = FIRST_BLOCK if (e == 0 and use_pre) else 0
                        for (t0, B) in make_blocks(caps[e], max_block, first):
                            if e == 0 and t0 == 0 and use_pre:
                                do_block(e, t0, B, xs0, w13f, w2f)
                                continue
                            xs = xp.tile([P, KO, max_block], BF16, tag="xs")
                            # x on its own DMA queue (gpsimd), chunked so
                            # the first chain only waits on the first chunk
                            for k0 in range(0, KO, 5):
                                nc.gpsimd.dma_start(
                                    xs[:, k0:k0 + 5, :B],
                                    xts[e][:, k0:k0 + 5, t0:t0 + B])
                            do_block(e, t0, B, xs)
                    if loop_reps and use_pre:
                        # refill the dedicated first-block buffers for the
                        # next iteration; lands well before the barrier.
                        load_first(xs0, w13f, w2f)
                if timing_mode:
                    db = op.tile([P, P], F32, tag="dumb")
                    nc.any.memset(db[:], 0.0)
                    nc.sync.dma_start(dum[:], db[:])
    nc.compile()
    return nc


def route(expert_indices, expert_weights, n_experts):
    """Per-expert token ids and merged combine weights (duplicates summed)."""
    idx = np.asarray(expert_indices)
    ew = np.asarray(expert_weights, dtype=np.float32)
    ids, wts = [], []
    for e in range(n_experts):
        m = idx == e                       # [T, K]
        rows = np.nonzero(m.any(axis=1))[0]
        ids.append(rows)
        wts.append((ew * m).sum(axis=1)[rows])
    return ids, wts


def balance(ids):
    """Pair k-th largest with k-th smallest expert by count.

    Returns (assign, caps): assign[core] = (slot0 expert, slot1 expert),
    caps = [C1, C2] padded slot capacities shared by all cores."""
    counts = np.array([len(i) for i in ids])
    order = np.argsort(-counts, kind="stable")
    assign = [(int(order[c]), int(order[E - 1 - c])) for c in range(NCORES)]
    pad = lambda n: max(((n + 15) // 16) * 16, 256)
    caps = [pad(int(counts[order[0]])), pad(int(counts[order[NCORES]]))]
    return assign, caps


def pack_weights(w1, w2, w3, dim=DIM, inter=INTER):
    """Pre-transpose weights to the device layout, cast bf16."""
    KO, MO = dim // P, inter // P
    w1b = np.asarray(w1).astype(ml_dtypes.bfloat16)
    w3b = np.asarray(w3).astype(ml_dtypes.bfloat16)
    w2b = np.asarray(w2).astype(ml_dtypes.bfloat16)
    ne = w1b.shape[0]
    # w1/w3: [e, INTER, DIM] -> [e, MO, P(k), KO, P(m)]
    w1p = w1b.reshape(ne, MO, P, KO, P).transpose(0, 1, 4, 3, 2).copy()
    w3p = w3b.reshape(ne, MO, P, KO, P).transpose(0, 1, 4, 3, 2).copy()
    # w2: [e, DIM, INTER] -> [e, KO, P(k over inter), MO, P(m over dim)]
    w2p = w2b.reshape(ne, KO, P, MO, P).transpose(0, 1, 4, 3, 2).copy()
    return w1p, w3p, w2p


def pack_tokens(x, ids, C, dim=DIM):
    """Gather routed tokens, pad to C, transpose to [P, KO, C] bf16."""
    KO = dim // P
    cnt = len(ids)
    xg = np.zeros((C, dim), dtype=np.float32)
    xg[:cnt] = np.asarray(x)[ids]
    xgb = xg.astype(ml_dtypes.bfloat16)
    return xgb.reshape(C, KO, P).transpose(2, 1, 0).copy()


def unpack_out(yt_e, C, dim=DIM):
    """[P, KO, C] -> [C, DIM] f32."""
    return yt_e.transpose(2, 1, 0).reshape(C, dim).astype(np.float32)


def kernel(x, expert_indices, expert_weights, w1, w2, w3):
    x = np.asarray(x, dtype=np.float32)
    w1 = np.asarray(w1, dtype=np.float32)
    w2 = np.asarray(w2, dtype=np.float32)
    w3 = np.asarray(w3, dtype=np.float32)

    ids, wts = route(expert_indices, expert_weights, E)
    assign, caps = balance(ids)

    nc = build_nc(caps)

    w1p, w3p, w2p = pack_weights(w1, w2, w3)
    in_maps = []
    for core in range(NCORES):
        exps = list(assign[core])
        im = {
            "w1t": w1p[exps].copy(),
            "w3t": w3p[exps].copy(),
            "w2t": w2p[exps].copy(),
        }
        for j, e in enumerate(exps):
            im[f"xt{j}"] = pack_tokens(x, ids[e], caps[j])
        in_maps.append(im)

    res = run_bass_kernel_spmd(nc, in_maps, core_ids=list(range(NCORES)))

    out = np.zeros((T, DIM), dtype=np.float32)
    for core in range(NCORES):
        for j, e in enumerate(assign[core]):
            cnt = len(ids[e])
            if cnt == 0:
                continue
            y = unpack_out(res.results[core][f"yt{j}"], caps[j])
            out[ids[e]] += wts[e][:, None] * y[:cnt]
    return out

